# revision 1
# baseline (speedup 1.0000x reference)
"""Trainium2 Bass kernel for a dense transformer block (pre-LN, causal MHA + GELU FFN).

Sharding: 8 cores = 4 batches x 2 roles. Each core handles one batch.
The two cores of a batch split the 2048 queries in a zigzag: role 0 owns
blocks [0:512) and [1536:2048), role 1 owns [512:1536). Both cores
redundantly compute LN1 + K/V for all 2048 tokens of their batch, which
avoids all cross-core communication. The causal structure is padded to a
common shape (8 k-tiles for the low query chunk, 16 for the high chunk)
and the per-role causal masks are host-provided data, so a single SPMD
program serves all cores.
"""

import time

import numpy as np
import ml_dtypes

import concourse.bass as bass
import concourse.tile as tile
from concourse import bacc
from concourse import mybir
from concourse.bass_utils import run_bass_kernel_spmd

F32 = mybir.dt.float32
BF16 = mybir.dt.bfloat16
AF = mybir.ActivationFunctionType
OP = mybir.AluOpType

B, S, E, H, DH = 4, 2048, 1024, 16, 64
MFF = 6 * E            # 6144
SO = S // 2            # own tokens per core: 1024
LN_EPS = 1e-5
NT = S // 128          # 16 token tiles (global)
NTO = SO // 128        # 8 own token tiles
NE = E // 128          # 8 feature chunks
NM = MFF // 128        # 48 ffn chunks
QC_KTILES = (8, 16)    # padded k-tile extents for the two query chunks

# debug toggles for HW bisection
import os
USE_PBCAST = True      # partition_broadcast + normalize in attention
MASK_ENGINE = "gpsimd"  # or "vector"
W2_DEPTH = int(os.environ.get("W2_DEPTH", "48"))


_prog_cache = {}


def _build_program(stage=4, reps=1):
    nc = bacc.Bacc(None)

    xg = nc.declare_dram_parameter("xg", [S, E], F32, isOutput=False)
    xo = nc.declare_dram_parameter("xo", [SO, E], F32, isOutput=False)
    wq = nc.declare_dram_parameter("wq", [E, E], BF16, isOutput=False)
    wk = nc.declare_dram_parameter("wk", [E, E], BF16, isOutput=False)
    wv = nc.declare_dram_parameter("wv", [E, E], BF16, isOutput=False)
    wo = nc.declare_dram_parameter("wo", [E, E], BF16, isOutput=False)
    w1 = nc.declare_dram_parameter("w1", [E, MFF], BF16, isOutput=False)
    w2 = nc.declare_dram_parameter("w2", [MFF, E], BF16, isOutput=False)
    bqk = nc.declare_dram_parameter("bqk", [128, 2, NE], F32, isOutput=False)
    b1d = nc.declare_dram_parameter("b1d", [128, NM], F32, isOutput=False)
    msk = nc.declare_dram_parameter("msk", [128, 8, 2048], BF16, isOutput=False)
    idn = nc.declare_dram_parameter("idn", [128, 128], BF16, isOutput=False)
    out = nc.declare_dram_parameter("out", [SO, E], F32, isOutput=True)

    with tile.TileContext(nc) as tc:
        def _body():
            # ---- kernel-wide pools ----
            gp = tc.alloc_tile_pool(name="gp", bufs=1)
            xin = tc.alloc_tile_pool(name="xin", bufs=2)
            stats = tc.alloc_tile_pool(name="stats", bufs=6)
            hrow = tc.alloc_tile_pool(name="hrow", bufs=2)

            masks = gp.tile([128, 8, 2048], BF16, tag="masks")
            ident = gp.tile([128, 128], BF16, tag="ident")
            bqk_s = gp.tile([128, 2, NE], F32, tag="bqk")
            b1_s = gp.tile([128, NM], F32, tag="b1")
            eps_t = gp.tile([128, 1], F32, tag="eps")

            nc.gpsimd.dma_start(out=ident, in_=idn[:, :])
            nc.gpsimd.dma_start(out=masks, in_=msk[:, :, :])
            nc.gpsimd.dma_start(out=bqk_s, in_=bqk[:, :, :])
            nc.gpsimd.dma_start(out=b1_s, in_=b1d[:, :])
            nc.vector.memset(eps_t, LN_EPS)

            dramp = tc.alloc_tile_pool(name="dramp", bufs=1, space="DRAM")

            def layernorm_tiles(src, ntiles, dstF, ps_tp, from_sbuf=False, dname="hd"):
                # LN per 128-token tile, spill normalized bf16 rows to DRAM,
                # then reload feature-major via DMA transpose (one per e-chunk).
                hd = dramp.tile([ntiles * 128, E], BF16, tag=dname, name=dname)
                for t in range(ntiles):
                    if from_sbuf:
                        xt = src[:, t, :]
                    else:
                        xt = xin.tile([128, E], F32, tag="xt", name=f"xt{t}")
                        nc.gpsimd.dma_start(out=xt, in_=src[t * 128:(t + 1) * 128, :])
                    st = stats.tile([128, 2, 6], F32, tag="st", name=f"st{t}")
                    nc.vector.bn_stats(out=st[:, 0, :], in_=xt[:, 0:512])
                    nc.vector.bn_stats(out=st[:, 1, :], in_=xt[:, 512:1024])
                    mv = stats.tile([128, 2], F32, tag="mv", name=f"mv{t}")
                    nc.vector.bn_aggr(out=mv, in_=st)
                    sd = stats.tile([128, 1], F32, tag="sd", name=f"sd{t}")
                    nc.scalar.activation(out=sd, in_=mv[:, 1:2], func=AF.Sqrt,
                                         bias=eps_t, scale=1.0)
                    rs = stats.tile([128, 1], F32, tag="rs", name=f"rs{t}")
                    nc.vector.reciprocal(out=rs, in_=sd)
                    ht = hrow.tile([128, E], BF16, tag="ht", name=f"ht{t}")
                    nc.vector.tensor_scalar(out=ht, in0=xt, scalar1=mv[:, 0:1],
                                            scalar2=rs, op0=OP.subtract, op1=OP.mult)
                    nc.gpsimd.dma_start(out=hd[t * 128:(t + 1) * 128, :], in_=ht)
                for e in range(NE):
                    nc.sync.dma_start(out=dstF[:, e, :],
                                      in_=hd[:, e * 128:(e + 1) * 128], transpose=True)

            # ============ phase A: LN1 + Q/K/V projections ============
            ab = tc.alloc_tile_pool(name="ab", bufs=1)
            KF = ab.tile([128, NE, S], BF16, tag="KF")
            QF = ab.tile([128, NE, SO], BF16, tag="QF")
            VT = ab.tile([128, NT, H * 65], BF16, tag="VT")

            ap = tc.alloc_tile_pool(name="ap", bufs=1)
            hF = ap.tile([128, NE, S], BF16, tag="hF")
            hFq = ap.tile([128, NE, SO], BF16, tag="hFq")
            wv_s = ap.tile([128, NE, E], BF16, tag="wv")
            wqkp = tc.alloc_tile_pool(name="wqkp", bufs=3)
            ps_tp = tc.alloc_tile_pool(name="ps_tp_a", bufs=2, space="PSUM")
            ps_mm = tc.alloc_tile_pool(name="ps_mm_a", bufs=6, space="PSUM")

            layernorm_tiles(xg, NT, hF, ps_tp, dname="hd1")
            layernorm_tiles(xo, NTO, hFq, ps_tp, dname="hdq")

            def proj_qk(w_dram, srcF, ntok, dstF, bias_col, pname):
                nch = ntok // 512
                for hp in range(NE):
                    pss = [ps_mm.tile([128, 512], F32, tag="mm", name=f"{pname}{hp}_{c}")
                           for c in range(nch)]
                    wt = wqkp.tile([128, NE, 128], BF16, tag="wqk",
                                   name=f"w{pname}{hp}")
                    nc.gpsimd.dma_start(
                        out=wt,
                        in_=w_dram[:, hp * 128:(hp + 1) * 128].rearrange(
                            "(e p) m -> p e m", p=128))
                    for e in range(NE):
                        for c in range(nch):
                            nc.tensor.matmul(
                                pss[c], wt[:, e, :], srcF[:, e, c * 512:(c + 1) * 512],
                                start=(e == 0), stop=(e == NE - 1))
                    for c in range(nch):
                        nc.any.tensor_scalar(
                            out=dstF[:, hp, c * 512:(c + 1) * 512], in0=pss[c],
                            scalar1=bqk_s[:, bias_col, hp:hp + 1], scalar2=None,
                            op0=OP.add)

            proj_qk(wq, hFq, SO, QF, 0, "q")
            proj_qk(wk, hF, S, KF, 1, "k")

            # V projection: token-major with a ones column per head
            for e in range(NE):
                nc.gpsimd.dma_start(out=wv_s[:, e, :], in_=wv[e * 128:(e + 1) * 128, :])
            VTv = VT.rearrange("p t (h c) -> p t h c", c=65)
            for t in range(NT):
                nc.vector.memset(VTv[:, t, :, 64:65], 1.0)
                for c in range(2):
                    ps = ps_mm.tile([128, 512], F32, tag="mm", name=f"v{t}_{c}")
                    for e in range(NE):
                        nc.tensor.matmul(
                            ps, hF[:, e, t * 128:(t + 1) * 128],
                            wv_s[:, e, c * 512:(c + 1) * 512],
                            start=(e == 0), stop=(e == NE - 1))
                    nc.any.tensor_copy(
                        out=VTv[:, t, 8 * c:8 * c + 8, 0:64],
                        in_=ps.rearrange("p (h c) -> p h c", c=64))

            ps_mm.release()
            ps_tp.release()
            wqkp.release()
            ap.release()

            # ============ phase B: attention ============
            skipB = stage < 2
            skipC = stage < 3
            skipD = stage < 4
            ct2p = tc.alloc_tile_pool(name="ct2p", bufs=1, side="right")
            CT2 = ct2p.tile([128, NE, SO], BF16, tag="CT2")
            if skipB:
                nc.vector.memset(CT2[:, :, :], 0.0)
            ptile = tc.alloc_tile_pool(name="ptile", bufs=3)
            small = tc.alloc_tile_pool(name="small", bufs=3)
            ps_sc = tc.alloc_tile_pool(name="ps_sc", bufs=1, space="PSUM")
            ps_ctx = tc.alloc_tile_pool(name="ps_ctx", bufs=4, space="PSUM")

            for hp in range(NE if not skipB else 0):
                for qc in range(2):
                    nkt = QC_KTILES[qc]
                    ng = nkt // 2  # groups of (2 k-tiles x 2 heads)
                    ctxs = (ps_ctx.tile([65, 512], F32, tag="ctx", name=f"cx{hp}_{qc}_0"),
                            ps_ctx.tile([65, 512], F32, tag="ctx", name=f"cx{hp}_{qc}_1"))
                    for g in range(ng):
                        sc = ps_sc.tile([128, 2048], F32, tag="sc", name=f"sc{hp}_{qc}_{g}")
                        for hh in range(2):
                            hoff = hh * 64
                            for kl in range(2):
                                kt = g * 2 + kl
                                nc.tensor.matmul(
                                    sc[:, (hh * 2 + kl) * 512:(hh * 2 + kl + 1) * 512],
                                    KF[hoff:hoff + 64, hp, kt * 128:(kt + 1) * 128],
                                    QF[hoff:hoff + 64, hp, qc * 512:(qc + 1) * 512],
                                    start=True, stop=True)
                        pt = ptile.tile([128, 2048], BF16, tag="pt", name=f"pt{hp}_{qc}_{g}")
                        nc.scalar.activation(out=pt, in_=sc, func=AF.Exp, scale=0.125)
                        # qc0: k-tiles 0..7 all need masks; qc1: only k-tiles
                        # 8..15 (groups 4..7) do.
                        if qc == 0 or g >= 4:
                            sub = g if qc == 0 else g - 4
                            slot = (0 if qc == 0 else 4) + sub
                            meng = nc.gpsimd if MASK_ENGINE == "gpsimd" else nc.vector
                            meng.tensor_tensor(out=pt, in0=pt,
                                               in1=masks[:, slot, :], op=OP.mult)
                        for hh in range(2):
                            h = hp * 2 + hh
                            for kl in range(2):
                                kt = g * 2 + kl
                                nc.tensor.matmul(
                                    ctxs[hh], VTv[:, kt, h, :],
                                    pt[:, (hh * 2 + kl) * 512:(hh * 2 + kl + 1) * 512],
                                    start=(g == 0 and kl == 0),
                                    stop=(g == ng - 1 and kl == 1))
                    for hh in range(2):
                        if USE_PBCAST:
                            rs1 = small.tile([1, 512], F32, tag="rs1", name=f"r{hp}_{qc}_{hh}")
                            nc.vector.reciprocal(out=rs1, in_=ctxs[hh][64:65, :])
                            rsb = small.tile([64, 512], F32, tag="rsb", name=f"rb{hp}_{qc}_{hh}")
                            nc.gpsimd.partition_broadcast(rsb, rs1)
                            nc.vector.tensor_tensor(
                                out=CT2[hh * 64:hh * 64 + 64, hp, qc * 512:(qc + 1) * 512],
                                in0=ctxs[hh][0:64, :], in1=rsb, op=OP.mult)
                        else:
                            nc.vector.tensor_copy(
                                out=CT2[hh * 64:hh * 64 + 64, hp, qc * 512:(qc + 1) * 512],
                                in_=ctxs[hh][0:64, :])

            ps_ctx.release()
            ps_sc.release()
            small.release()
            ptile.release()
            ab.release()

            # ============ phase C: Wo + residual, LN2, FFN up + gelu ============
            x2p = tc.alloc_tile_pool(name="x2p", bufs=1)
            X2 = x2p.tile([128, NTO, E], F32, tag="X2")
            wop = tc.alloc_tile_pool(name="wop", bufs=1)
            wo_s = wop.tile([128, NE, E], BF16, tag="wo")
            ps_tp2 = tc.alloc_tile_pool(name="ps_tp_c", bufs=2, space="PSUM")
            ps_mm2 = tc.alloc_tile_pool(name="ps_mm_c", bufs=4, space="PSUM")

            for e in range(NE):
                nc.gpsimd.dma_start(out=wo_s[:, e, :], in_=wo[e * 128:(e + 1) * 128, :])
            for qt in range(NTO):
                xot = xin.tile([128, E], F32, tag="xt", name=f"xo{qt}")
                nc.gpsimd.dma_start(out=xot, in_=xo[qt * 128:(qt + 1) * 128, :])
                for eo in range(2):
                    ps = ps_mm2.tile([128, 512], F32, tag="mm", name=f"o{qt}_{eo}")
                    for hp in range(NE):
                        nc.tensor.matmul(ps, CT2[:, hp, qt * 128:(qt + 1) * 128],
                                         wo_s[:, hp, eo * 512:(eo + 1) * 512],
                                         start=(hp == 0), stop=(hp == NE - 1))
                    nc.vector.tensor_tensor(
                        out=X2[:, qt, eo * 512:(eo + 1) * 512], in0=ps,
                        in1=xot[:, eo * 512:(eo + 1) * 512], op=OP.add)

            wop.release()
            ct2p.release()

            h2p = tc.alloc_tile_pool(name="h2p", bufs=1, side="right")
            h2F = h2p.tile([128, NE, SO], BF16, tag="h2F")
            if skipC:
                nc.vector.memset(h2F[:, :, :], 0.0)
            else:
                layernorm_tiles(X2, NTO, h2F, ps_tp2, from_sbuf=True, dname="hd2")

            hidp = tc.alloc_tile_pool(name="hidp", bufs=1)
            HID = hidp.tile([128, NM, SO], BF16, tag="HID")
            w1p = tc.alloc_tile_pool(name="w1p", bufs=3)
            for mo in range(NM if not skipC else 0):
                w1t = w1p.tile([128, NE, 128], BF16, tag="w1t", name=f"w1t{mo}")
                nc.gpsimd.dma_start(
                    out=w1t,
                    in_=w1[:, mo * 128:(mo + 1) * 128].rearrange("(e p) m -> p e m", p=128))
                for c in range(2):
                    ps = ps_mm2.tile([128, 512], F32, tag="mm", name=f"h{mo}_{c}")
                    for e in range(NE):
                        nc.tensor.matmul(
                            ps, w1t[:, e, :], h2F[:, e, c * 512:(c + 1) * 512],
                            start=(e == 0), stop=(e == NE - 1))
                    nc.scalar.activation(
                        out=HID[:, mo, c * 512:(c + 1) * 512], in_=ps, func=AF.Gelu,
                        bias=b1_s[:, mo:mo + 1], scale=1.0)

            w1p.release()
            h2p.release()
            ps_mm2.release()
            ps_tp2.release()

            # ============ phase D: FFN down + residual + store ============
            w2p = tc.alloc_tile_pool(name="w2p", bufs=3)
            outp = tc.alloc_tile_pool(name="outp", bufs=3)
            ps_f2 = tc.alloc_tile_pool(name="ps_f2", bufs=8, space="PSUM")
            if skipC or skipD:
                nc.vector.memset(HID[:, :, :], 0.0)
            for eo in range(2):
                pss = [ps_f2.tile([128, 512], F32, tag="f2", name=f"f{eo}_{j}")
                       for j in range(8)]
                for m in range((W2_DEPTH if not skipD else 1)):
                    w2t = w2p.tile([128, 512], BF16, tag="w2t", name=f"w2t{eo}_{m}")
                    nc.gpsimd.dma_start(
                        out=w2t, in_=w2[m * 128:(m + 1) * 128, eo * 512:(eo + 1) * 512])
                    for qt in range(8):
                        nc.tensor.matmul(
                            pss[qt], HID[:, m, qt * 128:(qt + 1) * 128], w2t,
                            start=(m == 0), stop=(m == NM - 1))
                for qt in range(8):
                    ot = outp.tile([128, 512], F32, tag="ot", name=f"ot{eo}_{qt}")
                    nc.vector.tensor_tensor(
                        out=ot, in0=pss[qt],
                        in1=X2[:, qt, eo * 512:(eo + 1) * 512], op=OP.add)
                    nc.gpsimd.dma_start(
                        out=out[qt * 128:(qt + 1) * 128, eo * 512:(eo + 1) * 512],
                        in_=ot)

            ps_f2.release()
            outp.release()
            w2p.release()
            hidp.release()
            x2p.release()
            hrow.release()
            stats.release()
            xin.release()
            gp.release()

        for _rep in range(reps):
            _body()

    nc.compile()
    return nc


def _own_slices(role):
    if role == 0:
        return [(0, 512), (1536, 2048)]
    return [(512, 1024), (1024, 1536)]


def _make_masks(role):
    """[128, 8, 2048] bf16; slot = qc*4 + k-tile-pair index; the pair's
    [128, 1024] mask is duplicated in both halves (one per head)."""
    qstarts = (0, 1536) if role == 0 else (512, 1024)
    m = np.zeros((128, 8, 2048), np.float32)
    ki = np.arange(128)[:, None]
    qi = np.arange(512)[None, :]
    for qc in range(2):
        qs = qstarts[qc]
        kt0 = 0 if qc == 0 else 8
        for sub in range(4):
            slot = qc * 4 + sub
            for kl in range(2):
                kt = kt0 + sub * 2 + kl
                blk = ((kt * 128 + ki) <= (qs + qi))
                m[:, slot, kl * 512:(kl + 1) * 512] = blk
                m[:, slot, 1024 + kl * 512:1024 + (kl + 1) * 512] = blk
    return m.astype(ml_dtypes.bfloat16)


def _prep_core_inputs(x, Wq, Wk, Wv, Wo, W1, W2, ln1_g, ln1_b, ln2_g, ln2_b):
    bf = ml_dtypes.bfloat16
    WqA = np.transpose(np.asarray(Wq, np.float32), (1, 0, 2)).reshape(E, E)
    WkA = np.transpose(np.asarray(Wk, np.float32), (1, 0, 2)).reshape(E, E)
    WvA = np.transpose(np.asarray(Wv, np.float32), (1, 0, 2)).reshape(E, E)
    g1 = np.asarray(ln1_g, np.float32)
    b1v = np.asarray(ln1_b, np.float32)
    g2 = np.asarray(ln2_g, np.float32)
    b2v = np.asarray(ln2_b, np.float32)
    assert np.all(b1v == 0.0), "nonzero ln1 bias unsupported (V bias path)"
    wq_d = (g1[:, None] * WqA).astype(bf)
    wk_d = (g1[:, None] * WkA).astype(bf)
    wv_d = (g1[:, None] * WvA).astype(bf)
    wo_d = np.asarray(Wo, np.float32).astype(bf)
    w1_d = (g2[:, None] * np.asarray(W1, np.float32)).astype(bf)
    w2_d = np.asarray(W2, np.float32).astype(bf)
    bq = b1v @ WqA
    bk = b1v @ WkA
    bqk = np.ascontiguousarray(
        np.stack([bq.reshape(NE, 128).T, bk.reshape(NE, 128).T], axis=1), np.float32)
    b1ff = b2v @ np.asarray(W1, np.float32)
    b1d = np.ascontiguousarray(b1ff.reshape(NM, 128).T, np.float32)
    idn = np.eye(128, dtype=bf)

    x = np.asarray(x, np.float32)
    in_maps = []
    for c in range(8):
        b, r = c // 2, c % 2
        xow = np.concatenate([x[b, s0:s1] for (s0, s1) in _own_slices(r)], axis=0)
        in_maps.append({
            "xg": np.ascontiguousarray(x[b]), "xo": np.ascontiguousarray(xow),
            "wq": wq_d, "wk": wk_d, "wv": wv_d, "wo": wo_d,
            "w1": w1_d, "w2": w2_d,
            "bqk": bqk, "b1d": b1d,
            "msk": _make_masks(r), "idn": idn,
        })
    return in_maps


def kernel(**inputs):
    if "prog" not in _prog_cache:
        _prog_cache["prog"] = _build_program()
    nc = _prog_cache["prog"]
    in_maps = _prep_core_inputs(**inputs)
    res = None
    last_err = None
    for attempt in range(3):
        try:
            res = run_bass_kernel_spmd(nc, in_maps, list(range(8)))
            break
        except Exception as e:  # transient device faults observed; retry
            last_err = e
            time.sleep(2.0)
    if res is None:
        raise last_err
    outs = res.results
    full = np.empty((B, S, E), np.float32)
    for c in range(8):
        b, r = c // 2, c % 2
        o = np.asarray(outs[c]["out"], np.float32)
        pos = 0
        for (s0, s1) in _own_slices(r):
            full[b, s0:s1] = o[pos:pos + (s1 - s0)]
            pos += s1 - s0
    return full



# revision 2
# speedup vs baseline: 620.8959x; 620.8959x over previous
"""Trainium2 Bass kernel v2 for the dense transformer block.

Same sharding as v1 (8 cores = 4 batches x 2 zigzag query-roles), with the
mechanics overhauled:
  - LN outputs transposed to feature-major via TensorE (identity matmul)
    straight out of SBUF -- no DRAM spill / DMA-transpose round trip.
  - wq/wk resident in SBUF via contiguous row-chunk DMAs (no 1024-descriptor
    SWDGE gathers inside the projection loop).
  - All HBM loads issued on the HWDGE (sync) ring, freeing GpSimd for the
    attention mask multiplies.
  - Attention scores PSUM split into [128,1024] tiles (2 banks) with bufs=2
    so score matmuls of tile g+1 overlap the exp/mask drain of tile g.
  - W1 streamed in 8-column-chunk batches, W2 in 4-row-chunk batches,
    double-buffered on the sync ring.
"""

import time

import numpy as np
import ml_dtypes

import concourse.bass as bass
import concourse.tile as tile
from concourse import bacc
from concourse import mybir
from concourse.bass_utils import run_bass_kernel_spmd

F32 = mybir.dt.float32
BF16 = mybir.dt.bfloat16
FP8 = mybir.dt.float8e4
DR = mybir.MatmulPerfMode.DoubleRow
AF = mybir.ActivationFunctionType
OP = mybir.AluOpType

S_W = 2.0 ** 10        # fp8 scale on qkv weights (host side)
S_H = 2.0 ** 4         # fp8 scale on ln1 output (device side)
DESCALE = 1.0 / (S_W * S_H)

B, S, E, H, DH = 4, 2048, 1024, 16, 64
MFF = 6 * E            # 6144
SO = S // 2            # own tokens per core: 1024
LN_EPS = 1e-5
NT = S // 128          # 16 token tiles (global)
NTO = SO // 128        # 8 own token tiles
NE = E // 128          # 8 feature chunks
NM = MFF // 128        # 48 ffn chunks
QC_KTILES = (8, 16)    # padded k-tile extents for the two query chunks

_prog_cache = {}


def _build_program(stage=4, reps=1):
    nc = bacc.Bacc(None)

    xg = nc.declare_dram_parameter("xg", [S, E], F32, isOutput=False)
    xo = nc.declare_dram_parameter("xo", [SO, E], F32, isOutput=False)
    wq = nc.declare_dram_parameter("wq", [E, E], FP8, isOutput=False)
    wk = nc.declare_dram_parameter("wk", [E, E], FP8, isOutput=False)
    wv = nc.declare_dram_parameter("wv", [E, E], FP8, isOutput=False)
    wo = nc.declare_dram_parameter("wo", [E, E], BF16, isOutput=False)
    w1 = nc.declare_dram_parameter("w1", [E, MFF], BF16, isOutput=False)
    w2 = nc.declare_dram_parameter("w2", [MFF, E], BF16, isOutput=False)
    bqk = nc.declare_dram_parameter("bqk", [128, 2, NE], F32, isOutput=False)
    b1d = nc.declare_dram_parameter("b1d", [128, NM], F32, isOutput=False)
    msk = nc.declare_dram_parameter("msk", [128, 16, 1024], BF16, isOutput=False)
    idn = nc.declare_dram_parameter("idn", [128, 128], BF16, isOutput=False)
    out = nc.declare_dram_parameter("out", [SO, E], F32, isOutput=True)

    with tile.TileContext(nc) as tc:
        def _body():
            # ---- kernel-wide pools ----
            gp = tc.alloc_tile_pool(name="gp", bufs=1)
            xin = tc.alloc_tile_pool(name="xin", bufs=3)
            stats = tc.alloc_tile_pool(name="stats", bufs=6)
            hrow = tc.alloc_tile_pool(name="hrow", bufs=3)

            ident = gp.tile([128, 128], BF16, tag="ident")
            bqk_s = gp.tile([128, 2, NE], F32, tag="bqk")
            b1_s = gp.tile([128, NM], F32, tag="b1")
            eps_t = gp.tile([128, 1], F32, tag="eps")

            nc.sync.dma_start(out=ident, in_=idn[:, :])
            nc.sync.dma_start(out=bqk_s, in_=bqk[:, :, :])
            nc.sync.dma_start(out=b1_s, in_=b1d[:, :])
            nc.vector.memset(eps_t, LN_EPS)

            def layernorm_tiles(src, ntiles, dstF, ps_tp, from_sbuf=False, pfx="h",
                                out_scale=None):
                # LN per 128-token tile; transpose to feature-major via PE.
                for t in range(ntiles):
                    if from_sbuf:
                        xt = src[:, t, :]
                    else:
                        xt = xin.tile([128, E], F32, tag="xt", name=f"{pfx}x{t}")
                        nc.sync.dma_start(out=xt, in_=src[t * 128:(t + 1) * 128, :])
                    st = stats.tile([128, 2, 6], F32, tag="st", name=f"{pfx}s{t}")
                    nc.vector.bn_stats(out=st[:, 0, :], in_=xt[:, 0:512])
                    nc.vector.bn_stats(out=st[:, 1, :], in_=xt[:, 512:1024])
                    mv = stats.tile([128, 2], F32, tag="mv", name=f"{pfx}m{t}")
                    nc.vector.bn_aggr(out=mv, in_=st)
                    sd = stats.tile([128, 1], F32, tag="sd", name=f"{pfx}d{t}")
                    nc.scalar.activation(out=sd, in_=mv[:, 1:2], func=AF.Sqrt,
                                         bias=eps_t, scale=1.0)
                    rs = stats.tile([128, 1], F32, tag="rs", name=f"{pfx}r{t}")
                    nc.vector.reciprocal(out=rs, in_=sd)
                    ht = hrow.tile([128, E], BF16, tag="ht", name=f"{pfx}t{t}")
                    nc.vector.tensor_scalar(out=ht, in0=xt, scalar1=mv[:, 0:1],
                                            scalar2=rs, op0=OP.subtract, op1=OP.mult)
                    pst = ps_tp.tile([128, NE, 128], BF16, tag="tp",
                                     name=f"{pfx}p{t}")
                    for e in range(NE):
                        nc.tensor.transpose(pst[:, e, :],
                                            ht[:, e * 128:(e + 1) * 128], ident)
                    if out_scale is None:
                        nc.vector.tensor_copy(out=dstF[:, :, t * 128:(t + 1) * 128],
                                              in_=pst)
                    else:
                        nc.vector.tensor_scalar(
                            out=dstF[:, :, t * 128:(t + 1) * 128], in0=pst,
                            scalar1=out_scale, scalar2=None, op0=OP.mult)

            # ============ phase A: LN1 + Q/K/V projections ============
            ab = tc.alloc_tile_pool(name="ab", bufs=1)
            KF = ab.tile([128, NE, S], BF16, tag="KF")
            QF = ab.tile([128, NE, SO], BF16, tag="QF")
            VT = ab.tile([128, NT, H * 65], BF16, tag="VT")

            ap = tc.alloc_tile_pool(name="ap", bufs=1)
            hF = ap.tile([128, NE, S], FP8, tag="hF")
            hFq = ap.tile([128, NE, SO], FP8, tag="hFq")
            wq_s = ap.tile([128, NE, E], FP8, tag="wq")
            wk_s = ap.tile([128, NE, E], FP8, tag="wk")
            wv_s = ap.tile([128, NE, E], FP8, tag="wv")
            ps_tp = tc.alloc_tile_pool(name="ps_tp_a", bufs=2, space="PSUM")
            ps_mm = tc.alloc_tile_pool(name="ps_mm_a", bufs=6, space="PSUM")

            for e in range(NE):
                nc.sync.dma_start(out=wq_s[:, e, :], in_=wq[e * 128:(e + 1) * 128, :])
                nc.sync.dma_start(out=wk_s[:, e, :], in_=wk[e * 128:(e + 1) * 128, :])
                nc.sync.dma_start(out=wv_s[:, e, :], in_=wv[e * 128:(e + 1) * 128, :])

            layernorm_tiles(xo, NTO, hFq, ps_tp, pfx="q", out_scale=S_H)
            layernorm_tiles(xg, NT, hF, ps_tp, pfx="g", out_scale=S_H)

            def proj_qk(w_sb, srcF, ntok, dstF, bias_col, pname):
                nch = ntok // 512
                for hp in range(NE):
                    pss = [ps_mm.tile([128, 512], F32, tag="mm", name=f"{pname}{hp}_{c}")
                           for c in range(nch)]
                    for ep in range(NE // 2):
                        for c in range(nch):
                            nc.tensor.matmul(
                                pss[c],
                                w_sb[:, 2 * ep:2 * ep + 2, hp * 128:(hp + 1) * 128],
                                srcF[:, 2 * ep:2 * ep + 2, c * 512:(c + 1) * 512],
                                start=(ep == 0), stop=(ep == NE // 2 - 1),
                                perf_mode=DR)
                    for c in range(nch):
                        nc.any.tensor_scalar(
                            out=dstF[:, hp, c * 512:(c + 1) * 512], in0=pss[c],
                            scalar1=DESCALE,
                            scalar2=bqk_s[:, bias_col, hp:hp + 1],
                            op0=OP.mult, op1=OP.add)

            proj_qk(wq_s, hFq, SO, QF, 0, "q")
            proj_qk(wk_s, hF, S, KF, 1, "k")

            # V projection: token-major with a ones column per head
            VTv = VT.rearrange("p t (h c) -> p t h c", c=65)
            for t in range(NT):
                nc.vector.memset(VTv[:, t, :, 64:65], 1.0)
                for c in range(2):
                    ps = ps_mm.tile([128, 512], F32, tag="mm", name=f"v{t}_{c}")
                    for ep in range(NE // 2):
                        nc.tensor.matmul(
                            ps, hF[:, 2 * ep:2 * ep + 2, t * 128:(t + 1) * 128],
                            wv_s[:, 2 * ep:2 * ep + 2, c * 512:(c + 1) * 512],
                            start=(ep == 0), stop=(ep == NE // 2 - 1),
                            perf_mode=DR)
                    nc.any.tensor_scalar(
                        out=VTv[:, t, 8 * c:8 * c + 8, 0:64],
                        in0=ps.rearrange("p (h c) -> p h c", c=64),
                        scalar1=DESCALE, scalar2=None, op0=OP.mult)

            ps_mm.release()
            ps_tp.release()
            ap.release()

            # ============ phase B: attention ============
            skipB = stage < 2
            skipC = stage < 3
            skipD = stage < 4
            ct2p = tc.alloc_tile_pool(name="ct2p", bufs=1, side="right")
            CT2 = ct2p.tile([128, NE, SO], BF16, tag="CT2")
            if skipB:
                nc.vector.memset(CT2[:, :, :], 0.0)
            mkp = tc.alloc_tile_pool(name="mkp", bufs=1)
            masks = mkp.tile([128, 16, 1024], BF16, tag="masks")
            nc.sync.dma_start(out=masks, in_=msk[:, :, :])
            ptile = tc.alloc_tile_pool(name="ptile", bufs=3)
            small = tc.alloc_tile_pool(name="small", bufs=3)
            ps_sc = tc.alloc_tile_pool(name="ps_sc", bufs=2, space="PSUM")
            ps_ctx = tc.alloc_tile_pool(name="ps_ctx", bufs=4, space="PSUM")

            for hp in range(NE if not skipB else 0):
                for qc in range(2):
                    nkt = QC_KTILES[qc]
                    ctxs = (ps_ctx.tile([65, 512], F32, tag="ctx", name=f"cx{hp}_{qc}_0"),
                            ps_ctx.tile([65, 512], F32, tag="ctx", name=f"cx{hp}_{qc}_1"))
                    for kt in range(nkt):
                        sc = ps_sc.tile([128, 1024], F32, tag="sc",
                                        name=f"sc{hp}_{qc}_{kt}")
                        for hh in range(2):
                            hoff = hh * 64
                            nc.tensor.matmul(
                                sc[:, hh * 512:(hh + 1) * 512],
                                KF[hoff:hoff + 64, hp, kt * 128:(kt + 1) * 128],
                                QF[hoff:hoff + 64, hp, qc * 512:(qc + 1) * 512],
                                start=True, stop=True)
                        pt = ptile.tile([128, 1024], BF16, tag="pt",
                                        name=f"pt{hp}_{qc}_{kt}")
                        nc.scalar.activation(out=pt, in_=sc, func=AF.Exp, scale=0.125)
                        # qc0: k-tiles 0..7 masked (slots 0..7);
                        # qc1: only k-tiles 8..15 (slots 8..15).
                        if qc == 0 or kt >= 8:
                            nc.vector.tensor_tensor(
                                out=pt, in0=pt, in1=masks[:, kt, :], op=OP.mult)
                        for hh in range(2):
                            h = hp * 2 + hh
                            nc.tensor.matmul(
                                ctxs[hh], VTv[:, kt, h, :],
                                pt[:, hh * 512:(hh + 1) * 512],
                                start=(kt == 0), stop=(kt == nkt - 1))
                    for hh in range(2):
                        rs1 = small.tile([1, 512], F32, tag="rs1", name=f"r{hp}_{qc}_{hh}")
                        nc.vector.reciprocal(out=rs1, in_=ctxs[hh][64:65, :])
                        rsb = small.tile([64, 512], F32, tag="rsb", name=f"rb{hp}_{qc}_{hh}")
                        nc.gpsimd.partition_broadcast(rsb, rs1)
                        nc.vector.tensor_tensor(
                            out=CT2[hh * 64:hh * 64 + 64, hp, qc * 512:(qc + 1) * 512],
                            in0=ctxs[hh][0:64, :], in1=rsb, op=OP.mult)

            ps_ctx.release()
            ps_sc.release()
            small.release()
            ptile.release()
            mkp.release()
            ab.release()

            # ============ phase C: Wo + residual, LN2, FFN up + gelu ============
            x2p = tc.alloc_tile_pool(name="x2p", bufs=1)
            X2 = x2p.tile([128, NTO, E], F32, tag="X2")
            wop = tc.alloc_tile_pool(name="wop", bufs=1)
            wo_s = wop.tile([128, NE, E], BF16, tag="wo")
            ps_tp2 = tc.alloc_tile_pool(name="ps_tp_c", bufs=2, space="PSUM")
            ps_mm2 = tc.alloc_tile_pool(name="ps_mm_c", bufs=4, space="PSUM")

            for e in range(NE):
                nc.sync.dma_start(out=wo_s[:, e, :], in_=wo[e * 128:(e + 1) * 128, :])
            for qt in range(NTO):
                xot = xin.tile([128, E], F32, tag="xt", name=f"xo{qt}")
                nc.sync.dma_start(out=xot, in_=xo[qt * 128:(qt + 1) * 128, :])
                for eo in range(2):
                    ps = ps_mm2.tile([128, 512], F32, tag="mm", name=f"o{qt}_{eo}")
                    for hp in range(NE):
                        nc.tensor.matmul(ps, CT2[:, hp, qt * 128:(qt + 1) * 128],
                                         wo_s[:, hp, eo * 512:(eo + 1) * 512],
                                         start=(hp == 0), stop=(hp == NE - 1))
                    nc.vector.tensor_tensor(
                        out=X2[:, qt, eo * 512:(eo + 1) * 512], in0=ps,
                        in1=xot[:, eo * 512:(eo + 1) * 512], op=OP.add)

            wop.release()
            ct2p.release()

            h2p = tc.alloc_tile_pool(name="h2p", bufs=1, side="right")
            h2F = h2p.tile([128, NE, SO], BF16, tag="h2F")
            if skipC:
                nc.vector.memset(h2F[:, :, :], 0.0)
            else:
                layernorm_tiles(X2, NTO, h2F, ps_tp2, from_sbuf=True, pfx="n")

            hidp = tc.alloc_tile_pool(name="hidp", bufs=1)
            HID = hidp.tile([128, NM, SO], BF16, tag="HID")
            w1p = tc.alloc_tile_pool(name="w1p", bufs=2)
            NB1 = 8            # w1 column-chunks per load batch
            for mb in range(NM // NB1 if not skipC else 0):
                w1t = w1p.tile([128, NE, NB1 * 128], BF16, tag="w1t", name=f"w1t{mb}")
                nc.sync.dma_start(
                    out=w1t,
                    in_=w1[:, mb * NB1 * 128:(mb + 1) * NB1 * 128].rearrange(
                        "(e p) m -> p e m", p=128))
                for mi in range(NB1):
                    mo = mb * NB1 + mi
                    for c in range(2):
                        ps = ps_mm2.tile([128, 512], F32, tag="mm", name=f"h{mo}_{c}")
                        for e in range(NE):
                            nc.tensor.matmul(
                                ps, w1t[:, e, mi * 128:(mi + 1) * 128],
                                h2F[:, e, c * 512:(c + 1) * 512],
                                start=(e == 0), stop=(e == NE - 1))
                        nc.scalar.activation(
                            out=HID[:, mo, c * 512:(c + 1) * 512], in_=ps, func=AF.Gelu,
                            bias=b1_s[:, mo:mo + 1], scale=1.0)

            w1p.release()
            h2p.release()
            ps_mm2.release()
            ps_tp2.release()

            # ============ phase D: FFN down + residual + store ============
            w2p = tc.alloc_tile_pool(name="w2p", bufs=3)
            outp = tc.alloc_tile_pool(name="outp", bufs=2)
            ps_f2 = tc.alloc_tile_pool(name="ps_f2", bufs=8, space="PSUM")
            if skipC or skipD:
                nc.vector.memset(HID[:, :, :], 0.0)
            NB2 = 4            # w2 row-chunks per load batch
            nb2 = (NM // NB2) if not skipD else 1
            for eo in range(2):
                pss = [ps_f2.tile([128, 512], F32, tag="f2", name=f"f{eo}_{j}")
                       for j in range(8)]
                for mb in range(nb2):
                    w2t = w2p.tile([128, NB2, 512], BF16, tag="w2t",
                                   name=f"w2t{eo}_{mb}")
                    nc.sync.dma_start(
                        out=w2t,
                        in_=w2[mb * NB2 * 128:(mb + 1) * NB2 * 128,
                               eo * 512:(eo + 1) * 512].rearrange(
                            "(j p) n -> p j n", p=128))
                    for ji in range(NB2):
                        m = mb * NB2 + ji
                        for qt in range(8):
                            nc.tensor.matmul(
                                pss[qt], HID[:, m, qt * 128:(qt + 1) * 128],
                                w2t[:, ji, :],
                                start=(m == 0), stop=(m == NM - 1))
                ot = outp.tile([128, NTO, 512], F32, tag="ot", name=f"ot{eo}")
                for qt in range(8):
                    nc.vector.tensor_tensor(
                        out=ot[:, qt, :], in0=pss[qt],
                        in1=X2[:, qt, eo * 512:(eo + 1) * 512], op=OP.add)
                nc.sync.dma_start(
                    out=out[:, eo * 512:(eo + 1) * 512].rearrange(
                        "(q p) n -> p q n", p=128),
                    in_=ot)

            ps_f2.release()
            outp.release()
            w2p.release()
            hidp.release()
            x2p.release()
            hrow.release()
            stats.release()
            xin.release()
            gp.release()

        for _rep in range(reps):
            _body()

    nc.compile()
    return nc


def _own_slices(role):
    if role == 0:
        return [(0, 512), (1536, 2048)]
    return [(512, 1024), (1024, 1536)]


def _make_masks(role):
    """[128, 16, 1024] bf16; slot = k-tile index (qc0: 0..7, qc1: 8..15);
    the [128, 512] mask is duplicated in both halves (one per head)."""
    qstarts = (0, 1536) if role == 0 else (512, 1024)
    m = np.zeros((128, 16, 1024), np.float32)
    ki = np.arange(128)[:, None]
    qi = np.arange(512)[None, :]
    for kt in range(16):
        qc = 0 if kt < 8 else 1
        qs = qstarts[qc]
        blk = ((kt * 128 + ki) <= (qs + qi))
        m[:, kt, 0:512] = blk
        m[:, kt, 512:1024] = blk
    return m.astype(ml_dtypes.bfloat16)


def _prep_core_inputs(x, Wq, Wk, Wv, Wo, W1, W2, ln1_g, ln1_b, ln2_g, ln2_b):
    bf = ml_dtypes.bfloat16
    WqA = np.transpose(np.asarray(Wq, np.float32), (1, 0, 2)).reshape(E, E)
    WkA = np.transpose(np.asarray(Wk, np.float32), (1, 0, 2)).reshape(E, E)
    WvA = np.transpose(np.asarray(Wv, np.float32), (1, 0, 2)).reshape(E, E)
    g1 = np.asarray(ln1_g, np.float32)
    b1v = np.asarray(ln1_b, np.float32)
    g2 = np.asarray(ln2_g, np.float32)
    b2v = np.asarray(ln2_b, np.float32)
    assert np.all(b1v == 0.0), "nonzero ln1 bias unsupported (V bias path)"
    f8 = ml_dtypes.float8_e4m3
    q8 = lambda w: np.clip(w * S_W, -240.0, 240.0).astype(f8)
    wq_d = q8(g1[:, None] * WqA)
    wk_d = q8(g1[:, None] * WkA)
    wv_d = q8(g1[:, None] * WvA)
    wo_d = np.asarray(Wo, np.float32).astype(bf)
    w1_d = (g2[:, None] * np.asarray(W1, np.float32)).astype(bf)
    w2_d = np.asarray(W2, np.float32).astype(bf)
    bq = b1v @ WqA
    bk = b1v @ WkA
    bqk_d = np.ascontiguousarray(
        np.stack([bq.reshape(NE, 128).T, bk.reshape(NE, 128).T], axis=1), np.float32)
    b1ff = b2v @ np.asarray(W1, np.float32)
    b1d_d = np.ascontiguousarray(b1ff.reshape(NM, 128).T, np.float32)
    idn_d = np.eye(128, dtype=bf)

    x = np.asarray(x, np.float32)
    in_maps = []
    for c in range(8):
        b, r = c // 2, c % 2
        xow = np.concatenate([x[b, s0:s1] for (s0, s1) in _own_slices(r)], axis=0)
        in_maps.append({
            "xg": np.ascontiguousarray(x[b]), "xo": np.ascontiguousarray(xow),
            "wq": wq_d, "wk": wk_d, "wv": wv_d, "wo": wo_d,
            "w1": w1_d, "w2": w2_d,
            "bqk": bqk_d, "b1d": b1d_d,
            "msk": _make_masks(r), "idn": idn_d,
        })
    return in_maps


def kernel(**inputs):
    if "prog" not in _prog_cache:
        _prog_cache["prog"] = _build_program()
    nc = _prog_cache["prog"]
    in_maps = _prep_core_inputs(**inputs)
    res = None
    last_err = None
    for attempt in range(3):
        try:
            res = run_bass_kernel_spmd(nc, in_maps, list(range(8)))
            break
        except Exception as e:  # transient device faults observed; retry
            last_err = e
            time.sleep(2.0)
    if res is None:
        raise last_err
    outs = res.results
    full = np.empty((B, S, E), np.float32)
    for c in range(8):
        b, r = c // 2, c % 2
        o = np.asarray(outs[c]["out"], np.float32)
        pos = 0
        for (s0, s1) in _own_slices(r):
            full[b, s0:s1] = o[pos:pos + (s1 - s0)]
            pos += s1 - s0
    return full


# revision 3
# speedup vs baseline: 711.2160x; 1.1455x over previous
"""Trainium2 Bass kernel v2 for the dense transformer block.

Same sharding as v1 (8 cores = 4 batches x 2 zigzag query-roles), with the
mechanics overhauled:
  - LN outputs transposed to feature-major via TensorE (identity matmul)
    straight out of SBUF -- no DRAM spill / DMA-transpose round trip.
  - wq/wk resident in SBUF via contiguous row-chunk DMAs (no 1024-descriptor
    SWDGE gathers inside the projection loop).
  - All HBM loads issued on the HWDGE (sync) ring, freeing GpSimd for the
    attention mask multiplies.
  - Attention scores PSUM split into [128,1024] tiles (2 banks) with bufs=2
    so score matmuls of tile g+1 overlap the exp/mask drain of tile g.
  - W1 streamed in 8-column-chunk batches, W2 in 4-row-chunk batches,
    double-buffered on the sync ring.
"""

import time

import numpy as np
import ml_dtypes

import concourse.bass as bass
import concourse.tile as tile
from concourse import bacc
from concourse import mybir
from concourse.bass_utils import run_bass_kernel_spmd

F32 = mybir.dt.float32
BF16 = mybir.dt.bfloat16
FP8 = mybir.dt.float8e4
DR = mybir.MatmulPerfMode.DoubleRow
AF = mybir.ActivationFunctionType
OP = mybir.AluOpType

S_W = 2.0 ** 10        # fp8 scale on qkv weights (host side)
S_H = 2.0 ** 4         # fp8 scale on ln1 output (device side)
DESCALE = 1.0 / (S_W * S_H)
S_V = 2.0 ** 5         # fp8 scale on V values; with the ones-col at 1.0 the
                       # normalized CT2 comes out as S_V * ctx (= S_C * ctx)
S_C = S_V              # fp8 scale of CT2 (context) values
DESC_WO = 1.0 / (S_C * S_W)

B, S, E, H, DH = 4, 2048, 1024, 16, 64
MFF = 6 * E            # 6144
SO = S // 2            # own tokens per core: 1024
LN_EPS = 1e-5
NT = S // 128          # 16 token tiles (global)
NTO = SO // 128        # 8 own token tiles
NE = E // 128          # 8 feature chunks
NM = MFF // 128        # 48 ffn chunks
QC_KTILES = (8, 16)    # padded k-tile extents for the two query chunks

_prog_cache = {}


def _build_program(stage=4, reps=1):
    nc = bacc.Bacc(None)

    xg = nc.declare_dram_parameter("xg", [S, E], F32, isOutput=False)
    wq = nc.declare_dram_parameter("wq", [E, E], FP8, isOutput=False)
    wk = nc.declare_dram_parameter("wk", [E, E], FP8, isOutput=False)
    wv = nc.declare_dram_parameter("wv", [E, E], FP8, isOutput=False)
    wo = nc.declare_dram_parameter("wo", [E, E], FP8, isOutput=False)
    w1 = nc.declare_dram_parameter("w1", [E, MFF], BF16, isOutput=False)
    w2 = nc.declare_dram_parameter("w2", [MFF, E], BF16, isOutput=False)
    bqk = nc.declare_dram_parameter("bqk", [128, 2, NE], F32, isOutput=False)
    b1d = nc.declare_dram_parameter("b1d", [128, NM], F32, isOutput=False)
    msk = nc.declare_dram_parameter("msk", [128, 16, 1024], BF16, isOutput=False)
    idn = nc.declare_dram_parameter("idn", [128, 128], BF16, isOutput=False)
    out = nc.declare_dram_parameter("out", [SO, E], F32, isOutput=True)

    with tile.TileContext(nc) as tc:
        def _body():
            # ---- kernel-wide pools ----
            gp = tc.alloc_tile_pool(name="gp", bufs=1)
            xin = tc.alloc_tile_pool(name="xin", bufs=3)
            stats = tc.alloc_tile_pool(name="stats", bufs=6)
            hrow = tc.alloc_tile_pool(name="hrow", bufs=3)

            ident = gp.tile([128, 128], BF16, tag="ident")
            bqk_s = gp.tile([128, 2, NE], F32, tag="bqk")
            b1_s = gp.tile([128, NM], F32, tag="b1")
            eps_t = gp.tile([128, 1], F32, tag="eps")
            esh_t = gp.tile([128, 1], F32, tag="esh")
            mkp = tc.alloc_tile_pool(name="mkp", bufs=1)
            masks = mkp.tile([128, 16, 1024], BF16, tag="masks")

            nc.scalar.dma_start(out=ident, in_=idn[:, :])
            nc.scalar.dma_start(out=bqk_s, in_=bqk[:, :, :])
            nc.scalar.dma_start(out=b1_s, in_=b1d[:, :])
            nc.scalar.dma_start(out=masks, in_=msk[:, :, :])
            nc.vector.memset(eps_t, LN_EPS)
            nc.vector.memset(esh_t, -2.5)

            def layernorm_tiles(src, ntiles, dstF, ps_tp, from_sbuf=False, pfx="h",
                                out_scale=None):
                # LN per 128-token tile; transpose to feature-major via PE.
                for t in range(ntiles):
                    if from_sbuf:
                        xt = src[:, t, :]
                    else:
                        xt = xin.tile([128, E], F32, tag="xt", name=f"{pfx}x{t}")
                        nc.sync.dma_start(out=xt, in_=src[t * 128:(t + 1) * 128, :])
                    st = stats.tile([128, 2, 6], F32, tag="st", name=f"{pfx}s{t}")
                    nc.vector.bn_stats(out=st[:, 0, :], in_=xt[:, 0:512])
                    nc.vector.bn_stats(out=st[:, 1, :], in_=xt[:, 512:1024])
                    mv = stats.tile([128, 2], F32, tag="mv", name=f"{pfx}m{t}")
                    nc.vector.bn_aggr(out=mv, in_=st)
                    sd = stats.tile([128, 1], F32, tag="sd", name=f"{pfx}d{t}")
                    nc.scalar.activation(out=sd, in_=mv[:, 1:2], func=AF.Sqrt,
                                         bias=eps_t, scale=1.0)
                    rs = stats.tile([128, 1], F32, tag="rs", name=f"{pfx}r{t}")
                    nc.vector.reciprocal(out=rs, in_=sd)
                    ht = hrow.tile([128, E], BF16, tag="ht", name=f"{pfx}t{t}")
                    nc.vector.tensor_scalar(out=ht, in0=xt, scalar1=mv[:, 0:1],
                                            scalar2=rs, op0=OP.subtract, op1=OP.mult)
                    pst = ps_tp.tile([128, NE, 128], BF16, tag="tp",
                                     name=f"{pfx}p{t}")
                    for e in range(NE):
                        nc.tensor.transpose(pst[:, e, :],
                                            ht[:, e * 128:(e + 1) * 128], ident)
                    if out_scale is None:
                        nc.vector.tensor_copy(out=dstF[:, :, t * 128:(t + 1) * 128],
                                              in_=pst)
                    else:
                        nc.vector.tensor_scalar(
                            out=dstF[:, :, t * 128:(t + 1) * 128], in0=pst,
                            scalar1=out_scale, scalar2=None, op0=OP.mult)

            # ============ phase A: LN1 + Q/K/V projections ============
            ab = tc.alloc_tile_pool(name="ab", bufs=1)
            KF = ab.tile([128, NE, S], BF16, tag="KF")
            QF = ab.tile([128, NE, SO], BF16, tag="QF")
            VT = ab.tile([128, NT, H * 65], BF16, tag="VT")

            ap = tc.alloc_tile_pool(name="ap", bufs=1)
            hF = ap.tile([128, NE, S], FP8, tag="hF")
            wq_s = ap.tile([128, NE, E], FP8, tag="wq")
            wk_s = ap.tile([128, NE, E], FP8, tag="wk")
            wv_s = ap.tile([128, NE, E], FP8, tag="wv")
            ps_tp = tc.alloc_tile_pool(name="ps_tp_a", bufs=2, space="PSUM")
            ps_mm = tc.alloc_tile_pool(name="ps_mm_a", bufs=6, space="PSUM")

            for e in range(NE):
                nc.scalar.dma_start(out=wq_s[:, e, :], in_=wq[e * 128:(e + 1) * 128, :])
                nc.scalar.dma_start(out=wk_s[:, e, :], in_=wk[e * 128:(e + 1) * 128, :])
                nc.scalar.dma_start(out=wv_s[:, e, :], in_=wv[e * 128:(e + 1) * 128, :])

            layernorm_tiles(xg, NT, hF, ps_tp, pfx="g", out_scale=S_H)

            def proj_qk(w_sb, srcF, ntok, dstF, bias_col, pname):
                nch = ntok // 512
                for hp in range(NE):
                    pss = [ps_mm.tile([128, 512], F32, tag="mm", name=f"{pname}{hp}_{c}")
                           for c in range(nch)]
                    for ep in range(NE // 2):
                        for c in range(nch):
                            nc.tensor.matmul(
                                pss[c],
                                w_sb[:, 2 * ep:2 * ep + 2, hp * 128:(hp + 1) * 128],
                                srcF[:, 2 * ep:2 * ep + 2, c * 512:(c + 1) * 512],
                                start=(ep == 0), stop=(ep == NE // 2 - 1),
                                perf_mode=DR)
                    for c in range(nch):
                        nc.any.tensor_scalar(
                            out=dstF[:, hp, c * 512:(c + 1) * 512], in0=pss[c],
                            scalar1=DESCALE,
                            scalar2=bqk_s[:, bias_col, hp:hp + 1],
                            op0=OP.mult, op1=OP.add)

            # own queries are the first SO tokens of the permuted layout
            proj_qk(wq_s, hF, SO, QF, 0, "q")
            proj_qk(wk_s, hF, S, KF, 1, "k")

            # V projection: token-major with a ones column per head
            VTv = VT.rearrange("p t (h c) -> p t h c", c=65)
            for t in range(NT):
                nc.vector.memset(VTv[:, t, :, 64:65], 1.0)
                for c in range(2):
                    ps = ps_mm.tile([128, 512], F32, tag="mm", name=f"v{t}_{c}")
                    for ep in range(NE // 2):
                        nc.tensor.matmul(
                            ps, hF[:, 2 * ep:2 * ep + 2, t * 128:(t + 1) * 128],
                            wv_s[:, 2 * ep:2 * ep + 2, c * 512:(c + 1) * 512],
                            start=(ep == 0), stop=(ep == NE // 2 - 1),
                            perf_mode=DR)
                    nc.any.tensor_scalar(
                        out=VTv[:, t, 8 * c:8 * c + 8, 0:64],
                        in0=ps.rearrange("p (h c) -> p h c", c=64),
                        scalar1=DESCALE * S_V, scalar2=None, op0=OP.mult)

            ps_mm.release()
            ps_tp.release()
            ap.release()

            # ============ phase B: attention ============
            skipB = stage < 2
            skipC = stage < 3
            skipD = stage < 4
            ct2p = tc.alloc_tile_pool(name="ct2p", bufs=1, side="right")
            CT2 = ct2p.tile([128, NE, SO], FP8, tag="CT2")
            if skipB:
                nc.vector.memset(CT2[:, :, :], 0.0)
            ptile = tc.alloc_tile_pool(name="ptile", bufs=3)
            small = tc.alloc_tile_pool(name="small", bufs=3)
            ps_sc = tc.alloc_tile_pool(name="ps_sc", bufs=2, space="PSUM")
            ps_ctx = tc.alloc_tile_pool(name="ps_ctx", bufs=4, space="PSUM")

            # permuted-key layout: k-tiles 0-7 = own tokens, 8-15 = the rest.
            # qc0 (own q 0-511): tiles {4-7, 12-15} are zero for both roles and
            # are skipped; the 8 computed tiles all carry masks (slots 0-7).
            # qc1 (own q 512-1023): all 16 tiles; {4-7, 12-15} masked
            # (slots 8-15), {0-3, 8-11} full for both roles.
            QC_KTS = ([0, 1, 2, 3, 8, 9, 10, 11], list(range(16)))
            MSLOT = {}
            for _i, _kt in enumerate(QC_KTS[0]):
                MSLOT[(0, _kt)] = _i
            for _i, _kt in enumerate([4, 5, 6, 7, 12, 13, 14, 15]):
                MSLOT[(1, _kt)] = 8 + _i
            for hp in range(NE if not skipB else 0):
                for qc in range(2):
                    kts = QC_KTS[qc]
                    npair = len(kts) // 2
                    ctxs = (ps_ctx.tile([65, 512], F32, tag="ctx", name=f"cx{hp}_{qc}_0"),
                            ps_ctx.tile([65, 512], F32, tag="ctx", name=f"cx{hp}_{qc}_1"))
                    nkt = len(kts)
                    for ki, kt in enumerate(kts):
                        sc = ps_sc.tile([128, 1024], F32, tag="sc",
                                        name=f"sc{hp}_{qc}_{kt}")
                        for hh in range(2):
                            hoff = hh * 64
                            nc.tensor.matmul(
                                sc[:, hh * 512:(hh + 1) * 512],
                                KF[hoff:hoff + 64, hp, kt * 128:(kt + 1) * 128],
                                QF[hoff:hoff + 64, hp, qc * 512:(qc + 1) * 512],
                                start=True, stop=True)
                        pt = ptile.tile([128, 1024], BF16, tag="pt",
                                        name=f"pt{hp}_{qc}_{kt}")
                        nc.scalar.activation(out=pt, in_=sc,
                                             func=AF.Exp, scale=0.125,
                                             bias=esh_t)
                        slot = MSLOT.get((qc, kt))
                        if slot is not None:
                            nc.vector.tensor_tensor(
                                out=pt, in0=pt,
                                in1=masks[:, slot, :], op=OP.mult)
                        for hh in range(2):
                            h = hp * 2 + hh
                            nc.tensor.matmul(
                                ctxs[hh], VTv[:, kt, h, :],
                                pt[:, hh * 512:(hh + 1) * 512],
                                start=(ki == 0), stop=(ki == nkt - 1))
                    for hh in range(2):
                        rs1 = small.tile([1, 512], F32, tag="rs1", name=f"r{hp}_{qc}_{hh}")
                        nc.vector.reciprocal(out=rs1, in_=ctxs[hh][64:65, :])
                        rsb = small.tile([64, 512], F32, tag="rsb", name=f"rb{hp}_{qc}_{hh}")
                        nc.gpsimd.partition_broadcast(rsb, rs1)
                        nc.vector.tensor_tensor(
                            out=CT2[hh * 64:hh * 64 + 64, hp, qc * 512:(qc + 1) * 512],
                            in0=ctxs[hh][0:64, :], in1=rsb, op=OP.mult)

            ps_ctx.release()
            ps_sc.release()
            small.release()
            ptile.release()
            ab.release()
            mkp.release()

            # ============ phase C: Wo + residual, LN2, FFN up + gelu ============
            x2p = tc.alloc_tile_pool(name="x2p", bufs=1)
            X2 = x2p.tile([128, NTO, E], F32, tag="X2")
            wop = tc.alloc_tile_pool(name="wop", bufs=1)
            wo_s = wop.tile([128, NE, E], FP8, tag="wo")
            ps_tp2 = tc.alloc_tile_pool(name="ps_tp_c", bufs=2, space="PSUM")
            ps_mm2 = tc.alloc_tile_pool(name="ps_mm_c", bufs=4, space="PSUM")

            for e in range(NE):
                nc.scalar.dma_start(out=wo_s[:, e, :], in_=wo[e * 128:(e + 1) * 128, :])
            for qt in range(NTO):
                xot = xin.tile([128, E], F32, tag="xt", name=f"xo{qt}")
                nc.sync.dma_start(out=xot, in_=xg[qt * 128:(qt + 1) * 128, :])
                for eo in range(2):
                    ps = ps_mm2.tile([128, 512], F32, tag="mm", name=f"o{qt}_{eo}")
                    for ep in range(NE // 2):
                        nc.tensor.matmul(
                            ps, CT2[:, 2 * ep:2 * ep + 2, qt * 128:(qt + 1) * 128],
                            wo_s[:, 2 * ep:2 * ep + 2, eo * 512:(eo + 1) * 512],
                            start=(ep == 0), stop=(ep == NE // 2 - 1),
                            perf_mode=DR)
                    nc.vector.tensor_scalar(
                        out=X2[:, qt, eo * 512:(eo + 1) * 512], in0=ps,
                        scalar1=DESC_WO, scalar2=None, op0=OP.mult)
                    nc.vector.tensor_tensor(
                        out=X2[:, qt, eo * 512:(eo + 1) * 512],
                        in0=X2[:, qt, eo * 512:(eo + 1) * 512],
                        in1=xot[:, eo * 512:(eo + 1) * 512], op=OP.add)

            wop.release()
            ct2p.release()

            h2p = tc.alloc_tile_pool(name="h2p", bufs=1, side="right")
            h2F = h2p.tile([128, NE, SO], BF16, tag="h2F")
            if skipC:
                nc.vector.memset(h2F[:, :, :], 0.0)
            else:
                layernorm_tiles(X2, NTO, h2F, ps_tp2, from_sbuf=True, pfx="n")

            hidp = tc.alloc_tile_pool(name="hidp", bufs=1)
            HID = hidp.tile([128, NM, SO], BF16, tag="HID")
            w1p = tc.alloc_tile_pool(name="w1p", bufs=2)
            NB1 = 8            # w1 column-chunks per load batch
            for mb in range(NM // NB1 if not skipC else 0):
                w1t = w1p.tile([128, NE, NB1 * 128], BF16, tag="w1t", name=f"w1t{mb}")
                nc.scalar.dma_start(
                    out=w1t,
                    in_=w1[:, mb * NB1 * 128:(mb + 1) * NB1 * 128].rearrange(
                        "(e p) m -> p e m", p=128))
                for mi in range(NB1):
                    mo = mb * NB1 + mi
                    for c in range(2):
                        ps = ps_mm2.tile([128, 512], F32, tag="mm", name=f"h{mo}_{c}")
                        for e in range(NE):
                            nc.tensor.matmul(
                                ps, w1t[:, e, mi * 128:(mi + 1) * 128],
                                h2F[:, e, c * 512:(c + 1) * 512],
                                start=(e == 0), stop=(e == NE - 1))
                        nc.scalar.activation(
                            out=HID[:, mo, c * 512:(c + 1) * 512], in_=ps, func=AF.Gelu,
                            bias=b1_s[:, mo:mo + 1], scale=1.0)

            w1p.release()
            h2p.release()
            ps_mm2.release()
            ps_tp2.release()

            # ============ phase D: FFN down + residual + store ============
            w2p = tc.alloc_tile_pool(name="w2p", bufs=3)
            outp = tc.alloc_tile_pool(name="outp", bufs=2)
            ps_f2 = tc.alloc_tile_pool(name="ps_f2", bufs=8, space="PSUM")
            if skipC or skipD:
                nc.vector.memset(HID[:, :, :], 0.0)
            NB2 = 4            # w2 row-chunks per load batch
            nb2 = (NM // NB2) if not skipD else 1
            for eo in range(2):
                pss = [ps_f2.tile([128, 512], F32, tag="f2", name=f"f{eo}_{j}")
                       for j in range(8)]
                for mb in range(nb2):
                    w2t = w2p.tile([128, NB2, 512], BF16, tag="w2t",
                                   name=f"w2t{eo}_{mb}")
                    nc.scalar.dma_start(
                        out=w2t,
                        in_=w2[mb * NB2 * 128:(mb + 1) * NB2 * 128,
                               eo * 512:(eo + 1) * 512].rearrange(
                            "(j p) n -> p j n", p=128))
                    for ji in range(NB2):
                        m = mb * NB2 + ji
                        for qt in range(8):
                            nc.tensor.matmul(
                                pss[qt], HID[:, m, qt * 128:(qt + 1) * 128],
                                w2t[:, ji, :],
                                start=(m == 0), stop=(m == NM - 1))
                ot = outp.tile([128, NTO, 512], F32, tag="ot", name=f"ot{eo}")
                for qt in range(8):
                    nc.vector.tensor_tensor(
                        out=ot[:, qt, :], in0=pss[qt],
                        in1=X2[:, qt, eo * 512:(eo + 1) * 512], op=OP.add)
                nc.sync.dma_start(
                    out=out[:, eo * 512:(eo + 1) * 512].rearrange(
                        "(q p) n -> p q n", p=128),
                    in_=ot)

            ps_f2.release()
            outp.release()
            w2p.release()
            hidp.release()
            x2p.release()
            hrow.release()
            stats.release()
            xin.release()
            gp.release()

        for _rep in range(reps):
            _body()

    nc.compile()
    return nc


def _own_slices(role):
    if role == 0:
        return [(0, 512), (1536, 2048)]
    return [(512, 1024), (1024, 1536)]


def _perm_positions(role):
    """Global positions of the permuted per-core token layout:
    own tokens (1024) first, then the remaining 1024 in order."""
    own = _own_slices(role)
    pos = [g for (s0, s1) in own for g in range(s0, s1)]
    own_set = set(pos)
    pos += [g for g in range(S) if g not in own_set]
    return np.asarray(pos, np.int64)


def _make_masks(role):
    """[128, 16, 1024] fp8; slots 0-7: qc0 masks for k-tiles [0-3, 8-11];
    slots 8-15: qc1 masks for k-tiles [4-7, 12-15]. The [128, 512] mask is
    duplicated in both halves (one per head). Causality is evaluated on
    global positions of the permuted layout."""
    pos = _perm_positions(role)
    m = np.zeros((128, 16, 1024), np.float32)
    ki = np.arange(128)
    qi = np.arange(512)
    slot_map = [(0, kt) for kt in [0, 1, 2, 3, 8, 9, 10, 11]] + \
               [(1, kt) for kt in [4, 5, 6, 7, 12, 13, 14, 15]]
    for s, (qc, kt) in enumerate(slot_map):
        gk = pos[kt * 128 + ki][:, None]
        gq = pos[qc * 512 + qi][None, :]
        blk = (gk <= gq)
        m[:, s, 0:512] = blk
        m[:, s, 512:1024] = blk
    return m.astype(ml_dtypes.bfloat16)


def _prep_core_inputs(x, Wq, Wk, Wv, Wo, W1, W2, ln1_g, ln1_b, ln2_g, ln2_b):
    bf = ml_dtypes.bfloat16
    WqA = np.transpose(np.asarray(Wq, np.float32), (1, 0, 2)).reshape(E, E)
    WkA = np.transpose(np.asarray(Wk, np.float32), (1, 0, 2)).reshape(E, E)
    WvA = np.transpose(np.asarray(Wv, np.float32), (1, 0, 2)).reshape(E, E)
    g1 = np.asarray(ln1_g, np.float32)
    b1v = np.asarray(ln1_b, np.float32)
    g2 = np.asarray(ln2_g, np.float32)
    b2v = np.asarray(ln2_b, np.float32)
    assert np.all(b1v == 0.0), "nonzero ln1 bias unsupported (V bias path)"
    f8 = ml_dtypes.float8_e4m3
    q8 = lambda w: np.clip(w * S_W, -240.0, 240.0).astype(f8)
    wq_d = q8(g1[:, None] * WqA)
    wk_d = q8(g1[:, None] * WkA)
    wv_d = q8(g1[:, None] * WvA)
    wo_d = q8(np.asarray(Wo, np.float32))
    w1_d = (g2[:, None] * np.asarray(W1, np.float32)).astype(bf)
    w2_d = np.asarray(W2, np.float32).astype(bf)
    bq = b1v @ WqA
    bk = b1v @ WkA
    bqk_d = np.ascontiguousarray(
        np.stack([bq.reshape(NE, 128).T, bk.reshape(NE, 128).T], axis=1), np.float32)
    b1ff = b2v @ np.asarray(W1, np.float32)
    b1d_d = np.ascontiguousarray(b1ff.reshape(NM, 128).T, np.float32)
    idn_d = np.eye(128, dtype=bf)

    x = np.asarray(x, np.float32)
    in_maps = []
    for c in range(8):
        b, r = c // 2, c % 2
        xperm = np.ascontiguousarray(x[b][_perm_positions(r)])
        in_maps.append({
            "xg": xperm,
            "wq": wq_d, "wk": wk_d, "wv": wv_d, "wo": wo_d,
            "w1": w1_d, "w2": w2_d,
            "bqk": bqk_d, "b1d": b1d_d,
            "msk": _make_masks(r), "idn": idn_d,
        })
    return in_maps


def kernel(**inputs):
    if "prog" not in _prog_cache:
        _prog_cache["prog"] = _build_program()
    nc = _prog_cache["prog"]
    in_maps = _prep_core_inputs(**inputs)
    res = None
    last_err = None
    for attempt in range(3):
        try:
            res = run_bass_kernel_spmd(nc, in_maps, list(range(8)))
            break
        except Exception as e:  # transient device faults observed; retry
            last_err = e
            time.sleep(2.0)
    if res is None:
        raise last_err
    outs = res.results
    full = np.empty((B, S, E), np.float32)
    for c in range(8):
        b, r = c // 2, c % 2
        o = np.asarray(outs[c]["out"], np.float32)
        pos = 0
        for (s0, s1) in _own_slices(r):
            full[b, s0:s1] = o[pos:pos + (s1 - s0)]
            pos += s1 - s0
    return full
